# revision 1
# baseline (speedup 1.0000x reference)
"""Trainium2 Bass kernel: Convpass adapter with hypernet-generated 3x3 conv.

Pipeline per core (data-parallel over batch, 8 images/core):
  hypernet: conv_w = emb @ w_hyper + b_hyper     (diag-window matmul trick)
  down:     x[pix,512] @ [w_down|w_down] -> [128, pix]  (xbar-transposed x;
            psum rows 0-63 == rows 64-127 so the conv can pack 2 taps)
  gelu1:    quickgelu(. + b_down), written twice: rows 0-63 at col+1 (dx=0
            taps), rows 64-127 at col (dx=1 taps) of a padded buffer
  conv:     3x3 as 3 K=128 matmuls (dx=0,1 packed) + 3 K=64 (dx=2)
  gelu2:    quickgelu(. * scale)
  up:       y @ w_up + b_up -> [pix, 512]        (ones-row fused bias)

Matmul inputs are bf16 (cast during SWDGE DMA load); accumulation is fp32 in
PSUM and the final up-projection result is stored/written in fp32.
"""

import numpy as np

import concourse.bass as bass
import concourse.mybir as mybir
import concourse.tile as tile
from concourse import bacc
from concourse.bass_utils import run_bass_kernel_spmd

# Problem shapes (hardcoded per contract).
B, H, W, C = 64, 28, 28, 512
DIM, EMB = 64, 64
NCORES = 8
B_LOC = B // NCORES            # 8 images per core
PIX = H * W                    # 784 pixels per image
PW = W + 2                     # 30 padded width
PAD = PW * (H + 2)             # 900 padded pixels per image
RH = 2                         # row-halves per image
RROWS = H // RH                # 14 rows per half
NHALF = RROWS * W              # 392 pixels per half-tile
PSUB = 112                     # pixel subtile (partition dim for transposes)
NSUB = PIX // PSUB             # 7 subtiles per image
KCH = C // 128                 # 4 contraction chunks of 128 channels
JTOT = DIM * DIM * 9           # 36864 hypernet outputs
OHALF = 32                     # o-channels per hypernet psum half
NHYP = OHALF * 9               # 288 = free size of hypernet matmuls

F32 = mybir.dt.float32
BF16 = mybir.dt.bfloat16
GELU_A = 1.702

_CACHE = {}


def build_kernel():
    if "nc" in _CACHE:
        return _CACHE["nc"]

    nc = bacc.Bacc("TRN2", target_bir_lowering=False, debug=False)

    x_d = nc.dram_tensor("x", [B_LOC * PIX, C], F32, kind="ExternalInput")
    wd_d = nc.dram_tensor("w_down", [C, DIM], F32, kind="ExternalInput")
    bd_d = nc.dram_tensor("b_down", [DIM], F32, kind="ExternalInput")
    wu_d = nc.dram_tensor("w_up", [DIM, C], F32, kind="ExternalInput")
    bu_d = nc.dram_tensor("b_up", [C], F32, kind="ExternalInput")
    sc_d = nc.dram_tensor("scale", [DIM], F32, kind="ExternalInput")
    emb_d = nc.dram_tensor("layer_emb", [EMB], F32, kind="ExternalInput")
    wh_d = nc.dram_tensor("w_hyper", [EMB, JTOT], F32, kind="ExternalInput")
    bh_d = nc.dram_tensor("b_hyper", [JTOT], F32, kind="ExternalInput")
    out_d = nc.dram_tensor("out", [B_LOC * PIX, C], F32, kind="ExternalOutput")

    with tile.TileContext(nc) as tc:
        with tc.tile_pool(name="consts", bufs=1) as consts:
            # ---- constants / small params ----
            # Identity for matmul-based transpose (regular matmul, not
            # transpose-mode: transpose-mode doesn't count as PE-busy for the
            # HAM clock monitor and keeps the PE throttled at 1.2 GHz).
            ident = consts.tile([PSUB, PSUB], BF16)
            from concourse.masks import make_identity

            make_identity(nc, ident[:])

            # w_down duplicated along M so the down matmul writes identical
            # psum halves: [128c, k, 128m] with m 0-63 == m 64-127.
            w_down2 = consts.tile([128, KCH, 128], BF16)
            for half in range(2):
                nc.gpsimd.dma_start(
                    w_down2[:, :, half * DIM : (half + 1) * DIM],
                    wd_d[:].rearrange("(k p) d -> p k d", p=128),
                )
            w_up65 = consts.tile([DIM + 1, C], BF16)
            nc.gpsimd.dma_start(w_up65[:DIM, :], wu_d[:])
            nc.gpsimd.dma_start(w_up65[DIM : DIM + 1, :], bu_d[:][None, :])

            b_down2 = consts.tile([128, 1], F32)
            for half in range(2):
                nc.sync.dma_start(
                    b_down2[half * DIM : (half + 1) * DIM, :], bd_d[:][:, None]
                )
            b_down_g2 = consts.tile([128, 1], F32)
            nc.vector.tensor_scalar_mul(b_down_g2[:], b_down2[:], GELU_A)

            scale_sb = consts.tile([DIM, 1], F32)
            nc.sync.dma_start(scale_sb[:], sc_d[:][:, None])
            scale_g = consts.tile([DIM, 1], F32)
            nc.vector.tensor_scalar_mul(scale_g[:], scale_sb[:], GELU_A)

            # ---- hypernet: W[i, o*9+t] = sum_e emb[e]*wh[e, o*576+i*9+t] + bh ----
            # T1: zeros except column 64 = emb; lhsT_i = T1[:, 64-i : 128-i]
            # places emb in output-partition i only.
            t1 = consts.tile([EMB, 128], BF16)
            nc.gpsimd.memset(t1[:], 0.0)
            nc.gpsimd.dma_start(t1[:, 64:65], emb_d[:][:, None])

            b_sb = consts.tile([DIM, DIM * 9], F32)
            with nc.allow_non_contiguous_dma(reason="tiny strided bias gather"):
                nc.sync.dma_start(
                    b_sb[:].rearrange("i (o t) -> i o t", t=9),
                    bh_d[:].rearrange("(o i t) -> i o t", o=DIM, i=DIM),
                )

            # rows 0-63: W[i, o*9+t]; rows 64-127: same shifted by one tap so
            # a K=128 conv matmul contracts taps (dy,0) and (dy,1) at once.
            w_conv2 = consts.tile([128, DIM * 9], BF16)

            with (
                tc.tile_pool(name="whpool", bufs=1) as whpool,
                tc.tile_pool(name="hpsum", bufs=1, space="PSUM") as hpsum,
            ):
                for h in range(2):
                    wh_sb = whpool.tile([EMB, JTOT // 2], BF16, tag="wh")
                    nc.gpsimd.dma_start(
                        wh_sb[:], wh_d[:, h * (JTOT // 2) : (h + 1) * (JTOT // 2)]
                    )
                    ps_w = hpsum.tile([DIM, 512], F32, tag="hp")
                    # [64e, 32o, 576j] view; per-i window = [o step 576][t step 1]
                    wh_v = wh_sb[:].rearrange("e (o j) -> e o j", j=DIM * 9)
                    for i in range(DIM):
                        nc.tensor.matmul(
                            ps_w[:, :NHYP],
                            t1[:, 64 - i : 128 - i],
                            wh_v[:, :, i * 9 : i * 9 + 9],
                            start=(i == 0),
                            stop=(i == DIM - 1),
                        )
                    nc.vector.tensor_tensor(
                        w_conv2[:DIM, h * NHYP : (h + 1) * NHYP],
                        ps_w[:, :NHYP],
                        b_sb[:, h * NHYP : (h + 1) * NHYP],
                        mybir.AluOpType.add,
                    )
            # bottom half = top shifted by one tap (partition move -> DMA)
            nc.sync.dma_start(
                w_conv2[DIM:, : DIM * 9 - 1], w_conv2[:DIM, 1 : DIM * 9]
            )
            nc.vector.memset(w_conv2[DIM:, DIM * 9 - 1 :], 0.0)
            w_conv_v = w_conv2[:].rearrange("i (o t) -> i o t", t=9)

            # ---- main per-image pipeline ----
            with (
                tc.tile_pool(name="xin", bufs=3) as xin,
                tc.tile_pool(name="xt", bufs=2) as xtp,
                tc.tile_pool(name="xact", bufs=3) as xactp,
                tc.tile_pool(name="yact", bufs=3) as yactp,
                tc.tile_pool(name="tmp", bufs=4) as tmpp,
                tc.tile_pool(name="outs", bufs=2) as outsp,
                tc.tile_pool(name="ps_t", bufs=2, space="PSUM") as ps_tp,
                tc.tile_pool(name="ps_s", bufs=4, space="PSUM") as ps_sp,
                tc.tile_pool(name="ps_u", bufs=2, space="PSUM") as ps_up,
            ):
                for img in range(B_LOC):
                    # 1) load x (cast fp32 -> bf16): [112, 7, 512]
                    x_sb = xin.tile([PSUB, NSUB, C], BF16, tag="x")
                    nc.gpsimd.dma_start(
                        x_sb[:],
                        x_d[img * PIX : (img + 1) * PIX, :].rearrange(
                            "(s p) c -> p s c", p=PSUB
                        ),
                    )

                    # 2) transpose to xT [128ch, 4k, 784pix] via PE matmuls
                    # against identity (out = x_chunk.T @ I in fp32 psum).
                    xT = xtp.tile([128, KCH, PIX], BF16, tag="xt")
                    for k in range(KCH):
                        for grp, (s0, ns) in enumerate(((0, 4), (4, 3))):
                            ps_x = ps_tp.tile([128, 448], F32, tag="pst")
                            for s in range(s0, s0 + ns):
                                nc.tensor.matmul(
                                    ps_x[:, (s - s0) * PSUB : (s - s0 + 1) * PSUB],
                                    x_sb[:, s, k * 128 : (k + 1) * 128],
                                    ident[:],
                                    start=True,
                                    stop=True,
                                )
                            cp = ps_x[:, : ns * PSUB]
                            dst = xT[:, k, s0 * PSUB : (s0 + ns) * PSUB]
                            if grp == 0:
                                nc.scalar.copy(dst, cp)
                            else:
                                nc.vector.tensor_copy(dst, cp)

                    # padded activation buffer [128, 30*30]; rows 64-127 hold
                    # the same activations shifted one column left.
                    x_act = xactp.tile([128, PAD], BF16, tag="xa")
                    nc.gpsimd.memset(x_act[:], 0.0)
                    x_act_v = x_act[:].rearrange("d (r c) -> d r c", c=PW)

                    y_act = yactp.tile([DIM + 1, PIX], BF16, tag="ya")
                    nc.vector.memset(y_act[DIM : DIM + 1, :], 1.0)

                    # 3) down-proj, k-outer over both halves -> psum [128, 392]
                    ps_ds = [ps_sp.tile([128, NHALF], F32, tag="pss", name=f"psd{img}_{rh}") for rh in range(RH)]
                    for k in range(KCH):
                        for rh in range(RH):
                            nc.tensor.matmul(
                                ps_ds[rh][:],
                                w_down2[:, k, :],
                                xT[:, k, rh * NHALF : (rh + 1) * NHALF],
                                start=(k == 0),
                                stop=(k == KCH - 1),
                            )
                    for rh in range(RH):
                        # 4) quickgelu -> padded interior (both row copies)
                        ps_d = ps_ds[rh]
                        t_t = tmpp.tile([128, NHALF], BF16, tag="t")
                        nc.vector.tensor_scalar_add(t_t[:], ps_d[:], b_down2[:])
                        s_t = tmpp.tile([128, NHALF], BF16, tag="s")
                        nc.scalar.activation(
                            s_t[:],
                            ps_d[:],
                            mybir.ActivationFunctionType.Sigmoid,
                            bias=b_down_g2[:],
                            scale=GELU_A,
                        )
                        rows = slice(1 + rh * RROWS, 1 + (rh + 1) * RROWS)
                        nc.vector.tensor_tensor(
                            x_act_v[:DIM, rows, 1 : 1 + W],
                            t_t[:DIM].rearrange("d (r c) -> d r c", c=W),
                            s_t[:DIM].rearrange("d (r c) -> d r c", c=W),
                            mybir.AluOpType.mult,
                        )
                        nc.vector.tensor_tensor(
                            x_act_v[DIM:, rows, 0:W],
                            t_t[DIM:].rearrange("d (r c) -> d r c", c=W),
                            s_t[DIM:].rearrange("d (r c) -> d r c", c=W),
                            mybir.AluOpType.mult,
                        )

                    for rh in range(RH):
                        # 5) conv: dx=0,1 packed (K=128) + dx=2 (K=64)
                        ps_c = ps_sp.tile([DIM, NHALF], F32, tag="pss")
                        first = True
                        for dy in range(3):
                            src = x_act_v[
                                :, rh * RROWS + dy : rh * RROWS + dy + RROWS, 0:W
                            ]
                            nc.tensor.matmul(
                                ps_c[:],
                                w_conv_v[:, :, dy * 3],
                                src,
                                start=first,
                                stop=False,
                            )
                            first = False
                        for dy in range(3):
                            src = x_act_v[
                                :DIM,
                                rh * RROWS + dy : rh * RROWS + dy + RROWS,
                                2 : 2 + W,
                            ]
                            nc.tensor.matmul(
                                ps_c[:],
                                w_conv_v[:DIM, :, dy * 3 + 2],
                                src,
                                start=False,
                                stop=(dy == 2),
                            )
                        # 6) quickgelu(scale * y)
                        t2 = tmpp.tile([DIM, NHALF], BF16, tag="t")
                        nc.vector.tensor_scalar_mul(t2[:], ps_c[:], scale_sb[:])
                        s2 = tmpp.tile([DIM, NHALF], BF16, tag="s")
                        nc.scalar.activation(
                            s2[:],
                            ps_c[:],
                            mybir.ActivationFunctionType.Sigmoid,
                            bias=0.0,
                            scale=scale_g[:],
                        )
                        nc.vector.tensor_tensor(
                            y_act[:DIM, rh * NHALF : (rh + 1) * NHALF],
                            t2[:],
                            s2[:],
                            mybir.AluOpType.mult,
                        )

                    # 7) up-proj + bias (ones row) -> [112, 7, 512] -> HBM
                    o_sb = outsp.tile([PSUB, NSUB, C], F32, tag="o")
                    for pt in range(NSUB):
                        ps_u = ps_up.tile([PSUB, C], F32, tag="psu")
                        nc.tensor.matmul(
                            ps_u[:],
                            y_act[:, pt * PSUB : (pt + 1) * PSUB],
                            w_up65[:],
                            start=True,
                            stop=True,
                        )
                        nc.scalar.copy(o_sb[:, pt, :], ps_u[:])
                    nc.scalar.dma_start(
                        out_d[img * PIX : (img + 1) * PIX, :].rearrange(
                            "(s p) c -> p s c", p=PSUB
                        ),
                        o_sb[:],
                    )

    nc.compile()
    _CACHE["nc"] = nc
    return nc


def _make_in_maps(inputs):
    x = np.ascontiguousarray(inputs["x"], dtype=np.float32)
    shared = {
        k: np.ascontiguousarray(inputs[k], np.float32)
        for k in (
            "w_down",
            "b_down",
            "w_up",
            "b_up",
            "scale",
            "layer_emb",
            "w_hyper",
            "b_hyper",
        )
    }
    in_maps = []
    for c in range(NCORES):
        xc = x[c * B_LOC : (c + 1) * B_LOC].reshape(B_LOC * PIX, C)
        in_maps.append({"x": np.ascontiguousarray(xc), **shared})
    return in_maps


def kernel(**inputs) -> np.ndarray:
    nc = build_kernel()
    in_maps = _make_in_maps(inputs)
    res = run_bass_kernel_spmd(nc, in_maps, core_ids=list(range(NCORES)))
    outs = [res.results[c]["out"].reshape(B_LOC, H, W, C) for c in range(NCORES)]
    return np.concatenate(outs, axis=0)


def run_traced(inputs, **kw):
    """For test.py: run with tracing to get HW exec time."""
    nc = build_kernel()
    in_maps = _make_in_maps(inputs)
    return run_bass_kernel_spmd(
        nc, in_maps, core_ids=list(range(NCORES)), trace=True, **kw
    )



# revision 2
# speedup vs baseline: 1.4463x; 1.4463x over previous
"""Trainium2 Bass kernel: Convpass adapter with hypernet-generated 3x3 conv.

Pipeline per core (data-parallel over batch, 8 images/core):
  hypernet: conv_w = emb @ w_hyper + b_hyper     (diag-window matmul trick,
            both j-halves packed on 128 partitions -> 64 matmuls of N=288)
  down:     xT[128c,4k,784] @ [w_down|w_down] -> psum [128, 392] per half
            (x arrives pre-transposed bf16 from the host; psum rows 0-63 ==
            rows 64-127 so the conv can pack 2 taps)
  gelu1:    quickgelu(. + b_down), written twice: rows 0-63 at col+1 (dx=0
            taps), rows 64-127 at col (dx=1 taps) of a padded buffer
  conv:     3x3 as 3 K=128 matmuls (dx=0,1 packed) + 3 K=64 (dx=2)
  gelu2:    quickgelu(. * scale)
  up:       out^T[128c,392] = w_up65[:,cslice].T @ y_act  (stationary w_up,
            ones-row fused bias); stored transposed bf16, host untransposes.

All matmul inputs are bf16 (x / w_hyper cast host-side); accumulation is fp32
in PSUM. The output crosses HBM as bf16 in [img, c, pix] layout and is
transposed + upcast to fp32 on the host.
"""

import numpy as np
import ml_dtypes

import concourse.bass as bass
import concourse.mybir as mybir
import concourse.tile as tile
from concourse import bacc
from concourse.bass_utils import run_bass_kernel_spmd

# Problem shapes (hardcoded per contract).
B, H, W, C = 64, 28, 28, 512
DIM, EMB = 64, 64
NCORES = 8
B_LOC = B // NCORES            # 8 images per core
PIX = H * W                    # 784 pixels per image
PW = W + 2                     # 30 padded width
PAD = PW * (H + 2)             # 900 padded pixels per image
RH = 2                         # row-halves per image
RROWS = H // RH                # 14 rows per half
NHALF = RROWS * W              # 392 pixels per half-tile
KCH = C // 128                 # 4 contraction chunks of 128 channels
JTOT = DIM * DIM * 9           # 36864 hypernet outputs
JHALF = JTOT // 2              # 18432 per packed e-half
NHYP = JHALF // DIM            # 288 = free size of packed hypernet matmuls

F32 = mybir.dt.float32
BF16 = mybir.dt.bfloat16
GELU_A = 1.702

_CACHE = {}


def build_kernel():
    if "nc" in _CACHE:
        return _CACHE["nc"]

    nc = bacc.Bacc("TRN2", target_bir_lowering=False, debug=False)

    x_d = nc.dram_tensor("x", [B_LOC, 128, KCH * PIX], BF16, kind="ExternalInput")
    wd_d = nc.dram_tensor("w_down", [C, DIM], F32, kind="ExternalInput")
    bd_d = nc.dram_tensor("b_down", [DIM], F32, kind="ExternalInput")
    wu_d = nc.dram_tensor("w_up", [DIM, C], F32, kind="ExternalInput")
    bu_d = nc.dram_tensor("b_up", [C], F32, kind="ExternalInput")
    sc_d = nc.dram_tensor("scale", [DIM], F32, kind="ExternalInput")
    emb_d = nc.dram_tensor("layer_emb", [EMB], F32, kind="ExternalInput")
    wh_d = nc.dram_tensor("w_hyper", [EMB, JTOT], BF16, kind="ExternalInput")
    bh_d = nc.dram_tensor("b_hyper", [JTOT], F32, kind="ExternalInput")
    out_d = nc.dram_tensor("out", [B_LOC, 128, KCH * PIX], BF16, kind="ExternalOutput")

    with tile.TileContext(nc) as tc:
        with tc.tile_pool(name="consts", bufs=1) as consts:
            # ---- constants / small params ----
            # w_down duplicated along M so the down matmul writes identical
            # psum halves: [128c, k, 128m] with m 0-63 == m 64-127.
            w_down2 = consts.tile([128, KCH, 128], BF16)
            for half in range(2):
                nc.gpsimd.dma_start(
                    w_down2[:, :, half * DIM : (half + 1) * DIM],
                    wd_d[:].rearrange("(k p) d -> p k d", p=128),
                )
            w_up65 = consts.tile([DIM + 1, C], BF16)
            nc.gpsimd.dma_start(w_up65[:DIM, :], wu_d[:])
            nc.gpsimd.dma_start(w_up65[DIM : DIM + 1, :], bu_d[:][None, :])

            b_down2 = consts.tile([128, 1], F32)
            for half in range(2):
                nc.sync.dma_start(
                    b_down2[half * DIM : (half + 1) * DIM, :], bd_d[:][:, None]
                )
            b_down_g2 = consts.tile([128, 1], F32)
            nc.vector.tensor_scalar_mul(b_down_g2[:], b_down2[:], GELU_A)

            scale_sb = consts.tile([DIM, 1], F32)
            nc.sync.dma_start(scale_sb[:], sc_d[:][:, None])
            scale_g = consts.tile([DIM, 1], F32)
            nc.vector.tensor_scalar_mul(scale_g[:], scale_sb[:], GELU_A)

            # ---- hypernet: W[i, o*9+t] = sum_e emb[e]*wh[e, o*576+i*9+t] + bh
            # Both j-halves packed on 128 partitions: rows 0-63 stream
            # wh[:, :JHALF], rows 64-127 stream wh[:, JHALF:]. t2 is zeros
            # except column 64 = [emb;0] and column 128 = [0;emb]; the window
            # t2[:, 64-i : 192-i] puts emb into output partitions i and 64+i.
            t2 = consts.tile([128, 192], BF16)
            nc.gpsimd.memset(t2[:], 0.0)
            nc.gpsimd.dma_start(t2[0:EMB, 64:65], emb_d[:][:, None])
            nc.gpsimd.dma_start(t2[EMB:128, 128:129], emb_d[:][:, None])

            # bias, pre-split to match the packed psum layout:
            # rows 0-63 = bh[i, o<32], rows 64-127 = bh[i, o>=32]
            b_sb2 = consts.tile([128, NHYP], F32)
            with nc.allow_non_contiguous_dma(reason="tiny strided bias gather"):
                for hl in range(2):
                    nc.sync.dma_start(
                        b_sb2[hl * DIM : (hl + 1) * DIM, :].rearrange(
                            "i (o t) -> i o t", t=9
                        ),
                        bh_d[hl * JHALF : (hl + 1) * JHALF].rearrange(
                            "(o i t) -> i o t", o=DIM // 2, i=DIM
                        ),
                    )

            # rows 0-63: W[i, o*9+t]; rows 64-127: same shifted by one tap so
            # a K=128 conv matmul contracts taps (dy,0) and (dy,1) at once.
            w_conv2 = consts.tile([128, DIM * 9], BF16)

            with (
                tc.tile_pool(name="whpool", bufs=1) as whpool,
                tc.tile_pool(name="hpsum", bufs=1, space="PSUM") as hpsum,
            ):
                wh_sb = whpool.tile([128, JHALF], BF16, tag="wh")
                for hl in range(2):
                    nc.scalar.dma_start(
                        wh_sb[hl * EMB : (hl + 1) * EMB, :],
                        wh_d[:, hl * JHALF : (hl + 1) * JHALF],
                    )
                ps_w = hpsum.tile([128, NHYP], F32, tag="hp")
                # [128e2, 32o, 576j] view; per-i window = [o step 576][t step 1]
                wh_v = wh_sb[:].rearrange("e (o j) -> e o j", j=DIM * 9)
                for i in range(DIM):
                    nc.tensor.matmul(
                        ps_w[:],
                        t2[:, 64 - i : 192 - i],
                        wh_v[:, :, i * 9 : i * 9 + 9],
                        start=(i == 0),
                        stop=(i == DIM - 1),
                    )
                w_tmp = whpool.tile([128, NHYP], BF16, tag="wtmp")
                nc.vector.tensor_tensor(
                    w_tmp[:], ps_w[:], b_sb2[:], mybir.AluOpType.add
                )
                # unpack: top half of w_conv2 = [rows 0-63 | rows 64-127]
                nc.vector.tensor_copy(w_conv2[:DIM, :NHYP], w_tmp[:DIM, :])
                nc.sync.dma_start(w_conv2[:DIM, NHYP:], w_tmp[DIM:, :])
            # bottom half = top shifted by one tap (partition move -> DMA)
            nc.sync.dma_start(
                w_conv2[DIM:, : DIM * 9 - 1], w_conv2[:DIM, 1 : DIM * 9]
            )
            nc.vector.memset(w_conv2[DIM:, DIM * 9 - 1 :], 0.0)
            w_conv_v = w_conv2[:].rearrange("i (o t) -> i o t", t=9)

            # ---- main per-image pipeline ----
            with (
                tc.tile_pool(name="xin", bufs=3) as xin,
                tc.tile_pool(name="xact", bufs=3) as xactp,
                tc.tile_pool(name="yact", bufs=3) as yactp,
                tc.tile_pool(name="tmp", bufs=4) as tmpp,
                tc.tile_pool(name="outs", bufs=2) as outsp,
                tc.tile_pool(name="ps_s", bufs=3, space="PSUM") as ps_sp,
                tc.tile_pool(name="ps_c", bufs=2, space="PSUM") as ps_cp,
                tc.tile_pool(name="ps_u", bufs=2, space="PSUM") as ps_up,
            ):
                for img in range(B_LOC):
                    # 1) load pre-transposed x: [128ch, 4k, 784pix] bf16
                    xT = xin.tile([128, KCH, PIX], BF16, tag="x")
                    nc.sync.dma_start(
                        xT[:].rearrange("p k n -> p (k n)"), x_d[img]
                    )

                    # padded activation buffer [128, 30*30]; rows 64-127 hold
                    # the same activations shifted one column left.
                    x_act = xactp.tile([128, PAD], BF16, tag="xa")
                    nc.gpsimd.memset(x_act[:], 0.0)
                    x_act_v = x_act[:].rearrange("d (r c) -> d r c", c=PW)

                    y_act = yactp.tile([DIM + 1, PIX], BF16, tag="ya")
                    nc.gpsimd.memset(y_act[DIM : DIM + 1, :], 1.0)

                    # 2) down-proj, k-outer over both halves -> psum [128, 392]
                    ps_ds = [
                        ps_sp.tile([128, NHALF], F32, tag="pss", name=f"psd{img}_{rh}")
                        for rh in range(RH)
                    ]
                    for k in range(KCH):
                        for rh in range(RH):
                            nc.tensor.matmul(
                                ps_ds[rh][:],
                                w_down2[:, k, :],
                                xT[:, k, rh * NHALF : (rh + 1) * NHALF],
                                start=(k == 0),
                                stop=(k == KCH - 1),
                            )
                    for rh in range(RH):
                        # 3) quickgelu -> padded interior (both row copies)
                        ps_d = ps_ds[rh]
                        t_t = tmpp.tile([128, NHALF], BF16, tag="t")
                        nc.vector.tensor_scalar_add(t_t[:], ps_d[:], b_down2[:])
                        s_t = tmpp.tile([128, NHALF], BF16, tag="s")
                        nc.scalar.activation(
                            s_t[:],
                            ps_d[:],
                            mybir.ActivationFunctionType.Sigmoid,
                            bias=b_down_g2[:],
                            scale=GELU_A,
                        )
                        rows = slice(1 + rh * RROWS, 1 + (rh + 1) * RROWS)
                        nc.vector.tensor_tensor(
                            x_act_v[:DIM, rows, 1 : 1 + W],
                            t_t[:DIM].rearrange("d (r c) -> d r c", c=W),
                            s_t[:DIM].rearrange("d (r c) -> d r c", c=W),
                            mybir.AluOpType.mult,
                        )
                        nc.vector.tensor_tensor(
                            x_act_v[DIM:, rows, 0:W],
                            t_t[DIM:].rearrange("d (r c) -> d r c", c=W),
                            s_t[DIM:].rearrange("d (r c) -> d r c", c=W),
                            mybir.AluOpType.mult,
                        )

                    for rh in range(RH):
                        # 4) conv: dx=0,1 packed (K=128) + dx=2 (K=64)
                        ps_c = ps_cp.tile([DIM, NHALF], F32, tag="psc")
                        first = True
                        for dy in range(3):
                            src = x_act_v[
                                :, rh * RROWS + dy : rh * RROWS + dy + RROWS, 0:W
                            ]
                            nc.tensor.matmul(
                                ps_c[:],
                                w_conv_v[:, :, dy * 3],
                                src,
                                start=first,
                                stop=False,
                            )
                            first = False
                        for dy in range(3):
                            src = x_act_v[
                                :DIM,
                                rh * RROWS + dy : rh * RROWS + dy + RROWS,
                                2 : 2 + W,
                            ]
                            nc.tensor.matmul(
                                ps_c[:],
                                w_conv_v[:DIM, :, dy * 3 + 2],
                                src,
                                start=False,
                                stop=(dy == 2),
                            )
                        # 5) quickgelu(scale * y)
                        t2s = tmpp.tile([DIM, NHALF], BF16, tag="t")
                        nc.vector.tensor_scalar_mul(t2s[:], ps_c[:], scale_sb[:])
                        s2 = tmpp.tile([DIM, NHALF], BF16, tag="s")
                        nc.scalar.activation(
                            s2[:],
                            ps_c[:],
                            mybir.ActivationFunctionType.Sigmoid,
                            bias=0.0,
                            scale=scale_g[:],
                        )
                        nc.vector.tensor_tensor(
                            y_act[:DIM, rh * NHALF : (rh + 1) * NHALF],
                            t2s[:],
                            s2[:],
                            mybir.AluOpType.mult,
                        )

                    # 6) up-proj + bias (ones row), transposed: for each
                    # 128-wide c-chunk, out^T[c, pix] = w_up65[:, cs].T @ y_act
                    o_sb = outsp.tile([128, KCH, PIX], BF16, tag="o")
                    for kc in range(KCH):
                        for rh in range(RH):
                            ps_u = ps_up.tile([128, NHALF], F32, tag="psu")
                            nc.tensor.matmul(
                                ps_u[:],
                                w_up65[:, kc * 128 : (kc + 1) * 128],
                                y_act[:, rh * NHALF : (rh + 1) * NHALF],
                                start=True,
                                stop=True,
                            )
                            dst = o_sb[:, kc, rh * NHALF : (rh + 1) * NHALF]
                            if (kc * RH + rh) % 2 == 0:
                                nc.scalar.copy(dst, ps_u[:])
                            else:
                                nc.vector.tensor_copy(dst, ps_u[:])
                    nc.scalar.dma_start(
                        out_d[img], o_sb[:].rearrange("p k n -> p (k n)")
                    )

    nc.compile()
    _CACHE["nc"] = nc
    return nc


def _make_in_maps(inputs):
    bf16 = ml_dtypes.bfloat16
    x = np.ascontiguousarray(inputs["x"], dtype=np.float32)
    shared = {
        k: np.ascontiguousarray(inputs[k], np.float32)
        for k in ("w_down", "b_down", "w_up", "b_up", "scale", "layer_emb", "b_hyper")
    }
    shared["w_hyper"] = np.ascontiguousarray(
        inputs["w_hyper"], np.float32
    ).astype(bf16)
    in_maps = []
    for c in range(NCORES):
        xc = x[c * B_LOC : (c + 1) * B_LOC].reshape(B_LOC, PIX, KCH, 128)
        xt = np.ascontiguousarray(xc.transpose(0, 3, 2, 1)).astype(bf16)
        in_maps.append(
            {"x": xt.reshape(B_LOC, 128, KCH * PIX), **shared}
        )
    return in_maps


def _untranspose_out(res):
    outs = []
    for c in range(NCORES):
        o = np.asarray(res.results[c]["out"]).reshape(B_LOC, 128, KCH, PIX)
        o = o.transpose(0, 3, 2, 1).astype(np.float32)  # [img, pix, kc, p]
        outs.append(o.reshape(B_LOC, H, W, C))
    return np.concatenate(outs, axis=0)


def kernel(**inputs) -> np.ndarray:
    nc = build_kernel()
    in_maps = _make_in_maps(inputs)
    res = run_bass_kernel_spmd(nc, in_maps, core_ids=list(range(NCORES)))
    return _untranspose_out(res)


def run_traced(inputs, **kw):
    """For test.py: run with tracing to get HW exec time."""
    nc = build_kernel()
    in_maps = _make_in_maps(inputs)
    return run_bass_kernel_spmd(
        nc, in_maps, core_ids=list(range(NCORES)), trace=True, **kw
    )
